# revision 9
# baseline (speedup 1.0000x reference)
"""Hebbian ABCD NN kernel for 8x Trainium2 NeuronCores.

Reference computation (per layer i in 0..2, sizes all 4096):
    h0 = tanh(W0 @ x + b0); h1 = tanh(W1 @ h0 + b1); y = W2 @ h1 + b2
    states = [x, h0, h1, sigmoid(y)]
    nW_i = 2 * A * (B*pre*post + C*pre + D*post + E),  A..E = H_i[...,0..4]
           pre = states[i] (len-4096 row broadcast), post = states[i+1] (col)
Outputs: (y, nW0, nW1, nW2).

Sharding: rows (`out` dim) of W_i/b_i/H_i split across 8 cores (512 rows each).
Each core computes its 512-row activation shard, AllGathers the full
activation between layers (2KB payloads), and streams its H shard
(126 MB) through elementwise DVE/ACT ops. Memory-bound on H traffic.
"""

import numpy as np

N = 4096
NCORES = 8
SHARD = N // NCORES        # 512 rows per core
P = 128                    # SBUF partitions
CHUNKS = SHARD // P        # 4 row-chunks of 128 per core
KT = 512                   # free-dim tile size for the Hebbian stream
NKT = N // KT

_CACHED_NC = None


def _build_nc(fwd=(0, 1, 2), heb=(0, 1, 2), no_cc=False, no_ttr=True):
    import concourse.bacc as bacc
    import concourse.mybir as mybir
    import concourse.tile as tile

    f32 = mybir.dt.float32
    AF = mybir.ActivationFunctionType
    OP = mybir.AluOpType

    nc = bacc.Bacc("TRN2", target_bir_lowering=False, debug=False,
                   num_devices=NCORES)

    x_t = nc.dram_tensor("x", [N], f32, kind="ExternalInput")
    W_t, b_t, H_t, nW_t = [], [], [], []
    for i in range(3):
        W_t.append(nc.dram_tensor(f"W{i}", [SHARD, N], f32, kind="ExternalInput"))
        b_t.append(nc.dram_tensor(f"b{i}", [SHARD], f32, kind="ExternalInput"))
        H_t.append(nc.dram_tensor(f"H{i}", [SHARD, N, 5], f32, kind="ExternalInput"))
        nW_t.append(nc.dram_tensor(f"nW{i}", [SHARD, N], f32, kind="ExternalOutput"))
    y_t = nc.dram_tensor("y", [SHARD], f32, kind="ExternalOutput")

    rg = [list(range(NCORES))]

    with tile.TileContext(nc) as tc:
        with (
            tc.tile_pool(name="pre", bufs=1) as pre_pool,
            tc.tile_pool(name="w", bufs=3) as w_pool,
            tc.tile_pool(name="hin", bufs=3) as h_pool,
            tc.tile_pool(name="tmp", bufs=2) as tmp_pool,
            tc.tile_pool(name="small", bufs=1) as small_pool,
            tc.tile_pool(name="dram", bufs=1, space="DRAM") as dram_pool,
        ):
            # Replicated pre-activation tiles, one per layer.
            pre_tiles = [pre_pool.tile([P, N], f32, tag=f"pre{i}", name=f"pre{i}") for i in range(3)]
            # pre0 = x broadcast to all 128 partitions.
            nc.sync.dma_start(pre_tiles[0][:, :],
                              x_t.ap().unsqueeze(0).to_broadcast([P, N]))

            # Bias tiles: b[c*128+p] -> btile[p, c]
            b_tiles = []
            for i in range(3):
                bt = small_pool.tile([P, CHUNKS], f32, tag=f"b{i}", name=f"bt{i}")
                nc.sync.dma_start(
                    bt[:, :], b_t[i].ap().rearrange("(c p) -> p c", p=P))
                b_tiles.append(bt)

            # post (and q) chunks per layer: [P,1] each
            post_chunks = [[None] * CHUNKS for _ in range(3)]

            def forward_layer(i):
                """matvec + activation for layer i; returns nothing, fills
                post_chunks[i] and (for i<2) pre_tiles[i+1] via AllGather."""
                act_dram = dram_pool.tile([SHARD], f32, tag=f"actd{i}", name=f"actd{i}")
                for c in range(CHUNKS):
                    wt = w_pool.tile([P, N], f32, tag="w", name="wt")
                    nc.sync.dma_start(wt[:, :], W_t[i].ap()[c * P:(c + 1) * P, :])
                    scratch = w_pool.tile([P, N], f32, tag="w", name="scratch")
                    acc = small_pool.tile([P, 1], f32, tag=f"acc{i}_{c}", name=f"acc{i}_{c}")
                    if no_ttr:
                        nc.vector.tensor_tensor(scratch[:, :], wt[:, :],
                                                pre_tiles[i][:, :], op=OP.mult)
                        nc.vector.tensor_reduce(
                            acc[:, :], scratch[:, :],
                            axis=mybir.AxisListType.X, op=OP.add)
                    else:
                        nc.vector.tensor_tensor_reduce(
                            out=scratch[:, :], in0=wt[:, :], in1=pre_tiles[i][:, :],
                            scale=1.0, scalar=0.0, op0=OP.mult, op1=OP.add,
                            accum_out=acc[:, :])
                    if i < 2:
                        h_c = small_pool.tile([P, 1], f32, tag=f"h{i}_{c}", name=f"h{i}_{c}")
                        nc.scalar.activation(h_c[:, :], acc[:, :], AF.Tanh,
                                             bias=b_tiles[i][:, c:c + 1], scale=1.0)
                        post_chunks[i][c] = h_c
                        nc.sync.dma_start(act_dram[c * P:(c + 1) * P], h_c[:, 0])
                    else:
                        y_c = small_pool.tile([P, 1], f32, tag=f"y_{c}", name=f"y_{c}")
                        nc.scalar.activation(y_c[:, :], acc[:, :], AF.Identity,
                                             bias=b_tiles[i][:, c:c + 1], scale=1.0)
                        nc.sync.dma_start(y_t.ap()[c * P:(c + 1) * P], y_c[:, 0])
                        s_c = small_pool.tile([P, 1], f32, tag=f"s_{c}", name=f"s_{c}")
                        nc.scalar.activation(s_c[:, :], y_c[:, :], AF.Sigmoid)
                        post_chunks[i][c] = s_c
                if i < 2 and not no_cc:
                    gat = dram_pool.tile([N], f32, tag=f"gat{i}", name=f"gat{i}",
                                         addr_space="Shared")
                    nc.gpsimd.collective_compute(
                        "AllGather", OP.bypass, replica_groups=rg,
                        ins=[act_dram.opt()], outs=[gat.opt()])
                    nc.sync.dma_start(
                        pre_tiles[i + 1][:, :],
                        gat[:].unsqueeze(0).to_broadcast([P, N]))

            def hebbian_layer(i):
                """stream H_i shard, emit nW_i = 2A*((B*pre+D)*post + (C*pre+E))"""
                for c in range(CHUNKS):
                    q = post_chunks[i][c]
                    for k in range(NKT):
                        k0 = k * KT
                        ht = h_pool.tile([P, KT * 5], f32, tag="hin", name="ht")
                        nc.sync.dma_start(
                            ht[:, :].rearrange("p (k f) -> p k f", f=5),
                            H_t[i].ap()[c * P:(c + 1) * P, k0:k0 + KT, :])
                        h3 = ht[:, :].rearrange("p (k f) -> p k f", f=5)
                        A_, B_, C_, D_, E_ = (h3[:, :, m] for m in range(5))
                        pre_k = pre_tiles[i][:, k0:k0 + KT]

                        t1 = tmp_pool.tile([P, KT], f32, tag="t1", name="t1")
                        nc.vector.tensor_tensor(t1[:, :], B_, pre_k, op=OP.mult)
                        t2 = tmp_pool.tile([P, KT], f32, tag="t2", name="t2")
                        nc.vector.tensor_tensor(t2[:, :], t1[:, :], D_, op=OP.add)
                        t3 = tmp_pool.tile([P, KT], f32, tag="t3", name="t3")
                        nc.scalar.activation(t3[:, :], t2[:, :], AF.Copy,
                                             scale=q[:, :])
                        t4 = tmp_pool.tile([P, KT], f32, tag="t4", name="t4")
                        nc.vector.tensor_tensor(t4[:, :], C_, pre_k, op=OP.mult)
                        t5 = tmp_pool.tile([P, KT], f32, tag="t5", name="t5")
                        nc.vector.tensor_tensor(t5[:, :], t4[:, :], E_, op=OP.add)
                        t6 = tmp_pool.tile([P, KT], f32, tag="t6", name="t6")
                        nc.vector.tensor_tensor(t6[:, :], t3[:, :], t5[:, :],
                                                op=OP.add)
                        a2 = tmp_pool.tile([P, KT], f32, tag="a2", name="a2")
                        nc.scalar.activation(a2[:, :], A_, AF.Copy, scale=2.0)
                        o = tmp_pool.tile([P, KT], f32, tag="o", name="o")
                        nc.vector.tensor_tensor(o[:, :], t6[:, :], a2[:, :],
                                                op=OP.mult)
                        nc.sync.dma_start(
                            nW_t[i].ap()[c * P:(c + 1) * P, k0:k0 + KT], o[:, :])

            for i in range(3):
                if i in fwd:
                    forward_layer(i)
                if i in heb and i in fwd:
                    hebbian_layer(i)

    nc.compile()
    return nc


def _get_nc():
    global _CACHED_NC
    if _CACHED_NC is None:
        _CACHED_NC = _build_nc()
    return _CACHED_NC


def shard_inputs(x, W0, b0, W1, b1, W2, b2, H0, H1, H2):
    Ws, bs, Hs = [W0, W1, W2], [b0, b1, b2], [H0, H1, H2]
    in_maps = []
    xs = np.ascontiguousarray(x, dtype=np.float32)
    for c in range(NCORES):
        sl = slice(c * SHARD, (c + 1) * SHARD)
        m = {"x": xs}
        for i in range(3):
            m[f"W{i}"] = np.ascontiguousarray(Ws[i][sl], dtype=np.float32)
            m[f"b{i}"] = np.ascontiguousarray(bs[i][sl], dtype=np.float32)
            m[f"H{i}"] = np.ascontiguousarray(Hs[i][sl], dtype=np.float32)
        in_maps.append(m)
    return in_maps


def kernel(x, W0, b0, W1, b1, W2, b2, H0, H1, H2, _trace=False, _tmpdir=None):
    from concourse import bass_utils

    nc = _get_nc()
    in_maps = shard_inputs(x, W0, b0, W1, b1, W2, b2, H0, H1, H2)
    res = bass_utils.run_bass_kernel_spmd(
        nc, in_maps, core_ids=list(range(NCORES)), trace=_trace,
        tmpdir=_tmpdir)
    rs = res.results
    y = np.concatenate([r["y"] for r in rs])
    nWs = [np.concatenate([r[f"nW{i}"] for r in rs], axis=0) for i in range(3)]
    if _trace:
        kernel.last_results = res
    return (y, nWs[0], nWs[1], nWs[2])
